# revision 4
# baseline (speedup 1.0000x reference)
"""MoE kernel for Trainium2 (8 NeuronCores, expert-parallel).

Contract: kernel(**inputs) takes FULL unsharded inputs, returns FULL output.

Strategy (per spec sharding hint): shard the E=8 expert weight stacks across
the 8 cores; route tokens to cores by top-k expert assignment (the host does
the gather/scatter as part of sharding, since inputs arrive full on host);
the tiny gate is computed on host CPU with the exact reference ops.

Device kernel (per core e): y = gelu(x_sel @ w1[e] + b1[e]) @ w2[e] + b2[e]
for that expert's selected tokens, computed in bf16 (fp32 accumulate) in
feature-major layout so no on-device transposes are needed:
  layer1: hT[h, t] += w1[d, h].T @ xT[d, t]   (lhsT = w1 tile, rhs = xT tile)
  layer2: yT[o, t] += w2[h, o].T @ hT[h, t]   (lhsT = w2 tile, rhs = hT tile)
Host then scatter-adds topw-weighted expert outputs into the full [N, O] out.
"""

import numpy as np
import ml_dtypes

N, D, O, E, TOPK = 8192, 1024, 1024, 8, 2
H = 2 * O
NCORES = 8

_BUILD_CACHE = {}
LAST_RESULTS = None  # BassKernelResults of the most recent device run


def _build(cap, use_b1, use_b2):
    import concourse.bass as bass  # noqa: F401
    import concourse.bacc as bacc
    import concourse.mybir as mybir
    from concourse import tile
    from contextlib import ExitStack

    fp32 = mybir.dt.float32
    bf16 = mybir.dt.bfloat16
    AF = mybir.ActivationFunctionType

    nc = bacc.Bacc(
        "TRN2", target_bir_lowering=False, debug=False, num_devices=NCORES
    )
    xt = nc.dram_tensor("xt", [D, cap], bf16, kind="ExternalInput")
    w1 = nc.dram_tensor("w1", [D, H], bf16, kind="ExternalInput")
    w2 = nc.dram_tensor("w2", [H, O], bf16, kind="ExternalInput")
    if use_b1:
        b1 = nc.dram_tensor("b1", [H, 1], fp32, kind="ExternalInput")
    if use_b2:
        b2 = nc.dram_tensor("b2", [O, 1], fp32, kind="ExternalInput")
    yt = nc.dram_tensor("yt", [O, cap], fp32, kind="ExternalOutput")

    PD, PH, PO = D // 128, H // 128, O // 128

    blocks = []
    t = 0
    while t < cap:
        tb = min(512, cap - t)
        blocks.append((t, tb))
        t += tb

    with ExitStack() as ctx:
        tc = ctx.enter_context(tile.TileContext(nc))
        wpool = ctx.enter_context(tc.tile_pool(name="w", bufs=1))
        xpool = ctx.enter_context(tc.tile_pool(name="x", bufs=2))
        hpool = ctx.enter_context(tc.tile_pool(name="h", bufs=2))
        ypool = ctx.enter_context(tc.tile_pool(name="y", bufs=3))
        pspool = ctx.enter_context(tc.tile_pool(name="ps", bufs=4, space="PSUM"))

        w1s = []
        for kd in range(PD):
            tw = wpool.tile([128, H], bf16, tag=f"w1_{kd}")
            nc.sync.dma_start(tw[:], w1[kd * 128:(kd + 1) * 128, :])
            w1s.append(tw)
        w2s = []
        for kh in range(PH):
            tw = wpool.tile([128, O], bf16, tag=f"w2_{kh}")
            nc.sync.dma_start(tw[:], w2[kh * 128:(kh + 1) * 128, :])
            w2s.append(tw)
        b1s = []
        if use_b1:
            for hb in range(PH):
                tb1 = wpool.tile([128, 1], fp32, tag=f"b1_{hb}")
                nc.sync.dma_start(tb1[:], b1[hb * 128:(hb + 1) * 128, :])
                b1s.append(tb1)
        b2s = []
        if use_b2:
            for ob in range(PO):
                tb2 = wpool.tile([128, 1], fp32, tag=f"b2_{ob}")
                nc.sync.dma_start(tb2[:], b2[ob * 128:(ob + 1) * 128, :])
                b2s.append(tb2)

        for (ts, tb) in blocks:
            xts = []
            for kd in range(PD):
                tx = xpool.tile([128, tb], bf16, tag=f"x_{kd}")
                nc.gpsimd.dma_start(tx[:], xt[kd * 128:(kd + 1) * 128, ts:ts + tb])
                xts.append(tx)

            hs = []
            for hb in range(PH):
                ps = pspool.tile([128, tb], fp32, tag="ps")
                for kd in range(PD):
                    nc.tensor.matmul(
                        ps[:],
                        w1s[kd][:, hb * 128:(hb + 1) * 128],
                        xts[kd][:],
                        start=(kd == 0),
                        stop=(kd == PD - 1),
                    )
                th = hpool.tile([128, tb], bf16, tag=f"h_{hb}")
                if use_b1:
                    nc.scalar.activation(th[:], ps[:], AF.Gelu, bias=b1s[hb][:])
                else:
                    nc.scalar.activation(th[:], ps[:], AF.Gelu)
                hs.append(th)

            for ob in range(PO):
                ps = pspool.tile([128, tb], fp32, tag="ps")
                for kh in range(PH):
                    nc.tensor.matmul(
                        ps[:],
                        w2s[kh][:, ob * 128:(ob + 1) * 128],
                        hs[kh][:],
                        start=(kh == 0),
                        stop=(kh == PH - 1),
                    )
                ty = ypool.tile([128, tb], fp32, tag="y")
                if use_b2:
                    nc.scalar.activation(ty[:], ps[:], AF.Copy, bias=b2s[ob][:])
                else:
                    nc.vector.tensor_copy(ty[:], ps[:])
                nc.sync.dma_start(yt[ob * 128:(ob + 1) * 128, ts:ts + tb], ty[:])

    nc.compile()
    return nc


def _gate_cpu(x, gw1, gb1, gw2, gb2):
    """Replicate the reference gate exactly (jax ops, CPU) -> topw, topi."""
    import jax
    import jax.numpy as jnp

    cpu = jax.devices("cpu")[0]
    with jax.default_device(cpu):
        xj = jnp.asarray(x)
        g = jax.nn.gelu(
            xj @ jnp.asarray(gw1) + jnp.asarray(gb1), approximate=False
        ) @ jnp.asarray(gw2) + jnp.asarray(gb2)
        gw = jax.nn.softmax(g, axis=-1)
        topw, topi = jax.lax.top_k(gw, TOPK)
        topw = topw / jnp.sum(topw, axis=-1, keepdims=True)
        return np.asarray(topw, np.float32), np.asarray(topi)


def kernel(x, gate_w1, gate_b1, gate_w2, gate_b2, w1, b1, w2, b2):
    global LAST_RESULTS
    from concourse.bass_utils import run_bass_kernel_spmd

    x = np.asarray(x, np.float32)
    w1 = np.asarray(w1, np.float32)
    b1 = np.asarray(b1, np.float32)
    w2 = np.asarray(w2, np.float32)
    b2 = np.asarray(b2, np.float32)

    topw, topi = _gate_cpu(
        x,
        np.asarray(gate_w1, np.float32),
        np.asarray(gate_b1, np.float32),
        np.asarray(gate_w2, np.float32),
        np.asarray(gate_b2, np.float32),
    )

    idxs, wgts = [], []
    for e in range(E):
        m0 = topi[:, 0] == e
        m1 = topi[:, 1] == e
        idx = np.nonzero(m0 | m1)[0]
        w = np.where(m0[idx], topw[idx, 0], topw[idx, 1]).astype(np.float32)
        idxs.append(idx)
        wgts.append(w)
    counts = [len(i) for i in idxs]
    cap = max(128, -(-max(counts) // 128) * 128)

    use_b1 = bool(np.any(b1))
    use_b2 = bool(np.any(b2))
    key = (cap, use_b1, use_b2)
    if key not in _BUILD_CACHE:
        _BUILD_CACHE[key] = _build(cap, use_b1, use_b2)
    nc = _BUILD_CACHE[key]

    bf = ml_dtypes.bfloat16
    in_maps = []
    for e in range(E):
        xte = np.zeros((D, cap), dtype=bf)
        xte[:, : counts[e]] = x[idxs[e]].T.astype(bf)
        im = {
            "xt": xte,
            "w1": np.ascontiguousarray(w1[e]).astype(bf),
            "w2": np.ascontiguousarray(w2[e]).astype(bf),
        }
        if use_b1:
            im["b1"] = np.ascontiguousarray(b1[e].reshape(H, 1), dtype=np.float32)
        if use_b2:
            im["b2"] = np.ascontiguousarray(b2[e].reshape(O, 1), dtype=np.float32)
        in_maps.append(im)

    LAST_RESULTS = run_bass_kernel_spmd(nc, in_maps, list(range(NCORES)))
    res = LAST_RESULTS.results

    out = np.zeros((N, O), np.float32)
    for e in range(E):
        ye = res[e]["yt"][:, : counts[e]].T
        out[idxs[e]] += wgts[e][:, None] * ye
    return out
